# revision 9
# baseline (speedup 1.0000x reference)
"""Causal depthwise conv1d (K=4) over packed ragged sequences + SiLU + conv-state
cache update, sharded channel-wise across 8 trn2 NeuronCores.

Strategy:
  - Channels (D=4096) sharded 512/core (tensor-parallel, per the module's tp
    logic). Host transposes x to channel-major (D, T) so each core DMAs
    contiguous rows; on-chip layout is [channels->partitions, tokens->free],
    so conv taps are free-dim offsets.
  - Per-core HBM floor (~67MB moved at ~300GB/s with all 8 cores saturating
    device HBM) is ~224us; no single compute engine covers 4 fp32 taps under
    that, so tap work is split by token tile across three engines:
      * PE tiles (F=512, 8 of 32 slots/group): 4 diagonal-matrix matmuls
        accumulate the taps in PSUM (fp32 matmul = 2-pass, ~4cyc/col).
      * DVE tiles (F=2048, 6 quads/group): tensor_scalar (2x mode) + 2
        scalar_tensor_tensor fused MACs on VectorE, final fused MAC on
        GpSimd.
    ScalarE runs silu(acc + bias) for every tile.
  - Sequence-boundary tokens (first 3 of each sequence, <= 27 rows) are
    recomputed exactly on the host; the conv-state cache update (1MB
    gather/scatter) is also metadata-sized and done on the host.
"""

import numpy as np

T = 16384
D = 4096
K = 4
NCORES = 8
DC = D // NCORES  # 512 channels per core
G = DC // 128     # 4 partition groups per core
HALO = K - 1      # 3

F_PE = 512        # PE tile (one fp32 PSUM bank)
F_DVE = 2048      # DVE tile

_cached_nc = None


def _schedule():
    """Per-group work list: [('pe'|'dve', t0), ...] covering T tokens.

    [PE, DVE-quad] x 6 + [PE, PE] = 8 PE tiles + 6 quads = 32 slots of 512.
    """
    items = []
    t = 0
    for _ in range(6):
        items.append(("pe", t))
        t += F_PE
        items.append(("dve", t))
        t += F_DVE
    while t < T:
        items.append(("pe", t))
        t += F_PE
    assert t == T
    return items


def _build_device_kernel():
    import concourse.bacc as bacc
    import concourse.mybir as mybir
    from concourse.masks import make_identity
    from concourse.tile import TileContext

    f32 = mybir.dt.float32
    mult = mybir.AluOpType.mult
    add = mybir.AluOpType.add
    silu_fn = mybir.ActivationFunctionType.Silu
    copy_fn = mybir.ActivationFunctionType.Copy

    nc = bacc.Bacc("TRN2", target_bir_lowering=False, debug=False,
                   num_devices=NCORES)

    xt = nc.dram_tensor("xt", [DC, T], f32, kind="ExternalInput")
    ws = nc.dram_tensor("ws", [128, G, K], f32, kind="ExternalInput")
    bs = nc.dram_tensor("bs", [128, G], f32, kind="ExternalInput")
    yt = nc.dram_tensor("yt", [DC, T], f32, kind="ExternalOutput")

    sched = _schedule()

    with TileContext(nc) as tc:
        with (
            tc.tile_pool(name="const", bufs=1) as cpool,
            tc.tile_pool(name="xpe", bufs=6) as xpe_pool,
            tc.tile_pool(name="ype", bufs=6) as ype_pool,
            tc.tile_pool(name="xdv", bufs=4) as xdv_pool,
            tc.tile_pool(name="tdv", bufs=4) as tdv_pool,
            tc.tile_pool(name="tg", bufs=3) as tg_pool,
            tc.tile_pool(name="ps", bufs=8, space="PSUM") as ppool,
        ):
            ws_sb = cpool.tile([128, G, K], f32)
            nc.sync.dma_start(out=ws_sb[:], in_=ws[:])
            bs_sb = cpool.tile([128, G], f32)
            nc.sync.dma_start(out=bs_sb[:], in_=bs[:])
            ident = cpool.tile([128, 128], f32)
            make_identity(nc, ident[:])
            wd_sb = cpool.tile([128, G, K, 128], f32)
            for g in range(G):
                for j in range(K):
                    nc.vector.tensor_scalar_mul(
                        wd_sb[:, g, j, :], ident[:], ws_sb[:, g, j:j + 1])

            for g in range(G):
                rows = slice(g * 128, (g + 1) * 128)
                for kind, t0 in sched:
                    F = F_PE if kind == "pe" else F_DVE
                    xtile = (xpe_pool if kind == "pe" else xdv_pool).tile(
                        [128, F + HALO], f32)
                    if t0 == 0:
                        nc.gpsimd.memset(xtile[:, 0:HALO], 0.0)
                        nc.sync.dma_start(out=xtile[:, HALO:], in_=xt[rows, 0:F])
                    else:
                        nc.sync.dma_start(
                            out=xtile[:], in_=xt[rows, t0 - HALO:t0 + F])

                    if kind == "pe":
                        ps = ppool.tile([128, F], f32)
                        for j in range(K):
                            # tap j: out[c,t] += w[c,j] * x[c, t-(K-1-j)]
                            nc.tensor.matmul(
                                ps[:], wd_sb[:, g, j, :], xtile[:, j:j + F],
                                start=(j == 0), stop=(j == K - 1),
                            )
                        ytile = ype_pool.tile([128, F], f32)
                        nc.scalar.activation(
                            ytile[:], ps[:], silu_fn,
                            bias=bs_sb[:, g:g + 1], scale=1.0)
                        nc.sync.dma_start(out=yt[rows, t0:t0 + F], in_=ytile[:])
                    else:
                        acc = tdv_pool.tile([128, F], f32)
                        t2 = tg_pool.tile([128, F], f32)
                        # tap 0 and tap 3 products on ScalarE (Copy + scale)
                        nc.scalar.activation(
                            acc[:], xtile[:, 0:F], copy_fn,
                            bias=0.0, scale=ws_sb[:, g, 0:1])
                        nc.scalar.activation(
                            t2[:], xtile[:, K - 1:K - 1 + F], copy_fn,
                            bias=0.0, scale=ws_sb[:, g, K - 1:K])
                        # taps 1, 2 as fused MACs on VectorE
                        for j in (1, 2):
                            nc.vector.scalar_tensor_tensor(
                                out=acc[:], in0=xtile[:, j:j + F],
                                scalar=ws_sb[:, g, j:j + 1], in1=acc[:],
                                op0=mult, op1=add)
                        # final combine on GpSimd
                        nc.gpsimd.tensor_add(acc[:], acc[:], t2[:])
                        nc.scalar.activation(
                            acc[:], acc[:], silu_fn,
                            bias=bs_sb[:, g:g + 1], scale=1.0)
                        nc.sync.dma_start(out=yt[rows, t0:t0 + F], in_=acc[:])

    nc.compile()
    return nc


def _get_nc():
    global _cached_nc
    if _cached_nc is None:
        _cached_nc = _build_device_kernel()
    return _cached_nc


def _silu(a):
    return a * (1.0 / (1.0 + np.exp(-a)))


def kernel(x, weight, bias, conv_state, seq_idx, conv_idx, state_ids,
           _run_opts=None):
    from concourse.bass_utils import run_bass_kernel_spmd

    x = np.asarray(x)
    weight = np.asarray(weight)
    bias = np.asarray(bias)
    conv_state = np.asarray(conv_state)
    seq_idx = np.asarray(seq_idx)
    conv_idx = np.asarray(conv_idx)
    state_ids = np.asarray(state_ids)

    x0 = x[0]                               # (T, D) f32
    w = weight[:, 0, :].astype(np.float32)  # (D, K)
    xT = np.ascontiguousarray(x0.T)         # (D, T)

    in_maps = []
    for c in range(NCORES):
        lo = c * DC
        w_core = w[lo:lo + DC]              # (DC, K)
        wsm = np.ascontiguousarray(
            w_core.reshape(G, 128, K).transpose(1, 0, 2))   # (128, G, K)
        bsm = np.ascontiguousarray(
            bias[lo:lo + DC].astype(np.float32).reshape(G, 128).T)  # (128, G)
        in_maps.append({
            "xt": np.ascontiguousarray(xT[lo:lo + DC]),
            "ws": wsm,
            "bs": bsm,
        })

    nc = _get_nc()
    run_opts = _run_opts or {}
    res = run_bass_kernel_spmd(nc, in_maps, core_ids=list(range(NCORES)),
                               **run_opts)

    outT = np.concatenate([r["yt"] for r in res.results], axis=0)  # (D, T)
    out = np.ascontiguousarray(outT.T)[None]                       # (1, T, D)

    # --- host fixup: first K-1 tokens of every sequence (exact recompute) ---
    starts = np.concatenate([[0], np.flatnonzero(np.diff(seq_idx) != 0) + 1])
    fix = (starts[:, None] + np.arange(HALO)[None]).ravel()
    fix = np.unique(fix[fix < T])
    if fix.size:
        acc = np.broadcast_to(bias.astype(np.float32), (fix.size, D)).copy()
        for j in range(K):
            s = K - 1 - j
            tm = fix - s
            tm_c = np.clip(tm, 0, T - 1)
            valid = (tm >= 0) & (seq_idx[tm_c] == seq_idx[fix])
            acc += np.where(valid[:, None], x0[tm_c], 0.0) * w[None, :, j]
        out[0, fix] = _silu(acc)

    # --- conv-state cache update (gather last-K rows, scatter into pool) ---
    new_conv_state = conv_state.copy()
    new_conv_state[state_ids] = np.transpose(x0[conv_idx], (0, 2, 1))

    if _run_opts is not None:
        return (out, new_conv_state), res
    return out, new_conv_state


# revision 10
# speedup vs baseline: 1.1450x; 1.1450x over previous
"""Causal depthwise conv1d (K=4) over packed ragged sequences + SiLU + conv-state
cache update, sharded channel-wise across 8 trn2 NeuronCores.

Strategy:
  - Channels (D=4096) sharded 512/core (tensor-parallel, per the module's tp
    logic). Host transposes x to channel-major (D, T) so each core DMAs
    contiguous rows; on-chip layout is [channels->partitions, tokens->free],
    so conv taps are free-dim offsets.
  - HBM efficiency depends strongly on per-partition row length (2KB rows:
    ~250GB/s, 32KB rows: ~395GB/s with all 8 cores). So IO moves in
    [128, 8192] megatiles (32KB rows): one DMA in, one DMA out per half-group.
  - No single engine covers 4 fp32 taps under the HBM floor (~170us/core):
    fp32 PE matmul is 2-pass (~4cyc/col), fp32 DVE tensor-ops are 1x. Work is
    split by token range within each megatile:
      * 4 PE tiles (512 tok): 4 diagonal-matrix matmuls accumulate taps in
        PSUM; ScalarE silu(psum+bias) into the output megatile.
      * 3 DVE quads (2048 tok): ScalarE seeds tap0 (Copy with per-channel
        scale), VectorE chains 3 scalar_tensor_tensor fused MACs in-place,
        ScalarE silu in-place.
  - Sequence-boundary tokens (first 3 of each sequence, <= 27 rows) are
    recomputed exactly on the host; the conv-state cache update (1MB
    gather/scatter) is metadata-sized and also done on the host.
"""

import numpy as np

T = 16384
D = 4096
K = 4
NCORES = 8
DC = D // NCORES  # 512 channels per core
G = DC // 128     # 4 partition groups per core
HALO = K - 1      # 3

F_IO = 8192       # megatile tokens (32KB rows)
F_PE = 512        # PE tile (one fp32 PSUM bank)
F_DVE = 2048      # DVE quad
N_PE_HALF = 4     # PE tiles per megatile; rest is DVE quads
N_DVE_HALF = (F_IO - N_PE_HALF * F_PE) // F_DVE  # 3

_cached_nc = None


def _build_device_kernel():
    import concourse.bacc as bacc
    import concourse.mybir as mybir
    from concourse.masks import make_identity
    from concourse.tile import TileContext

    f32 = mybir.dt.float32
    mult = mybir.AluOpType.mult
    add = mybir.AluOpType.add
    silu_fn = mybir.ActivationFunctionType.Silu
    copy_fn = mybir.ActivationFunctionType.Copy

    nc = bacc.Bacc("TRN2", target_bir_lowering=False, debug=False,
                   num_devices=NCORES)

    xt = nc.dram_tensor("xt", [DC, T], f32, kind="ExternalInput")
    ws = nc.dram_tensor("ws", [128, G, K], f32, kind="ExternalInput")
    bs = nc.dram_tensor("bs", [128, G], f32, kind="ExternalInput")
    yt = nc.dram_tensor("yt", [DC, T], f32, kind="ExternalOutput")

    with TileContext(nc) as tc:
        with (
            tc.tile_pool(name="const", bufs=1) as cpool,
            tc.tile_pool(name="xb", bufs=2) as xpool,
            tc.tile_pool(name="yb", bufs=2) as ypool,
            tc.tile_pool(name="ps", bufs=8, space="PSUM") as ppool,
        ):
            ws_sb = cpool.tile([128, G, K], f32)
            nc.sync.dma_start(out=ws_sb[:], in_=ws[:])
            bs_sb = cpool.tile([128, G], f32)
            nc.sync.dma_start(out=bs_sb[:], in_=bs[:])
            ident = cpool.tile([128, 128], f32)
            make_identity(nc, ident[:])
            wd_sb = cpool.tile([128, G, K, 128], f32)
            for g in range(G):
                for j in range(K):
                    nc.vector.tensor_scalar_mul(
                        wd_sb[:, g, j, :], ident[:], ws_sb[:, g, j:j + 1])

            for g in range(G):
                rows = slice(g * 128, (g + 1) * 128)
                for t0 in range(0, T, F_IO):
                    xb = xpool.tile([128, F_IO + HALO], f32)
                    if t0 == 0:
                        nc.gpsimd.memset(xb[:, 0:HALO], 0.0)
                        nc.sync.dma_start(out=xb[:, HALO:], in_=xt[rows, 0:F_IO])
                    else:
                        nc.sync.dma_start(
                            out=xb[:], in_=xt[rows, t0 - HALO:t0 + F_IO])
                    yb = ypool.tile([128, F_IO], f32)

                    # --- DVE quads first (ACT seeds them early) ---
                    for q in range(N_DVE_HALF):
                        u = N_PE_HALF * F_PE + q * F_DVE
                        acc = yb[:, u:u + F_DVE]
                        nc.scalar.activation(
                            acc, xb[:, u:u + F_DVE], copy_fn,
                            bias=0.0, scale=ws_sb[:, g, 0:1])
                        for j in (1, 2, 3):
                            nc.vector.scalar_tensor_tensor(
                                out=acc, in0=xb[:, u + j:u + j + F_DVE],
                                scalar=ws_sb[:, g, j:j + 1], in1=acc,
                                op0=mult, op1=add)
                        nc.scalar.activation(
                            acc, acc, silu_fn,
                            bias=bs_sb[:, g:g + 1], scale=1.0)

                    # --- PE tiles (consecutive, so matmul passes pipeline) ---
                    for p in range(N_PE_HALF):
                        u = p * F_PE
                        ps = ppool.tile([128, F_PE], f32)
                        for j in range(K):
                            # tap j: out[c,t] += w[c,j] * x[c, t-(K-1-j)]
                            nc.tensor.matmul(
                                ps[:], wd_sb[:, g, j, :],
                                xb[:, u + j:u + j + F_PE],
                                start=(j == 0), stop=(j == K - 1),
                            )
                        nc.scalar.activation(
                            yb[:, u:u + F_PE], ps[:], silu_fn,
                            bias=bs_sb[:, g:g + 1], scale=1.0)

                    nc.sync.dma_start(out=yt[rows, t0:t0 + F_IO], in_=yb[:])

    nc.compile()
    return nc


def _get_nc():
    global _cached_nc
    if _cached_nc is None:
        _cached_nc = _build_device_kernel()
    return _cached_nc


def _silu(a):
    return a * (1.0 / (1.0 + np.exp(-a)))


def kernel(x, weight, bias, conv_state, seq_idx, conv_idx, state_ids,
           _run_opts=None):
    from concourse.bass_utils import run_bass_kernel_spmd

    x = np.asarray(x)
    weight = np.asarray(weight)
    bias = np.asarray(bias)
    conv_state = np.asarray(conv_state)
    seq_idx = np.asarray(seq_idx)
    conv_idx = np.asarray(conv_idx)
    state_ids = np.asarray(state_ids)

    x0 = x[0]                               # (T, D) f32
    w = weight[:, 0, :].astype(np.float32)  # (D, K)
    xT = np.ascontiguousarray(x0.T)         # (D, T)

    in_maps = []
    for c in range(NCORES):
        lo = c * DC
        w_core = w[lo:lo + DC]              # (DC, K)
        wsm = np.ascontiguousarray(
            w_core.reshape(G, 128, K).transpose(1, 0, 2))   # (128, G, K)
        bsm = np.ascontiguousarray(
            bias[lo:lo + DC].astype(np.float32).reshape(G, 128).T)  # (128, G)
        in_maps.append({
            "xt": np.ascontiguousarray(xT[lo:lo + DC]),
            "ws": wsm,
            "bs": bsm,
        })

    nc = _get_nc()
    run_opts = _run_opts or {}
    res = run_bass_kernel_spmd(nc, in_maps, core_ids=list(range(NCORES)),
                               **run_opts)

    outT = np.concatenate([r["yt"] for r in res.results], axis=0)  # (D, T)
    out = np.ascontiguousarray(outT.T)[None]                       # (1, T, D)

    # --- host fixup: first K-1 tokens of every sequence (exact recompute) ---
    starts = np.concatenate([[0], np.flatnonzero(np.diff(seq_idx) != 0) + 1])
    fix = (starts[:, None] + np.arange(HALO)[None]).ravel()
    fix = np.unique(fix[fix < T])
    if fix.size:
        acc = np.broadcast_to(bias.astype(np.float32), (fix.size, D)).copy()
        for j in range(K):
            s = K - 1 - j
            tm = fix - s
            tm_c = np.clip(tm, 0, T - 1)
            valid = (tm >= 0) & (seq_idx[tm_c] == seq_idx[fix])
            acc += np.where(valid[:, None], x0[tm_c], 0.0) * w[None, :, j]
        out[0, fix] = _silu(acc)

    # --- conv-state cache update (gather last-K rows, scatter into pool) ---
    new_conv_state = conv_state.copy()
    new_conv_state[state_ids] = np.transpose(x0[conv_idx], (0, 2, 1))

    if _run_opts is not None:
        return (out, new_conv_state), res
    return out, new_conv_state


# revision 11
# speedup vs baseline: 1.2118x; 1.0584x over previous
"""Causal depthwise conv1d (K=4) over packed ragged sequences + SiLU + conv-state
cache update, sharded channel-wise across 8 trn2 NeuronCores.

Strategy:
  - Channels (D=4096) sharded 512/core (tensor-parallel, per the module's tp
    logic). Host transposes x to channel-major (D, T) so each core DMAs
    contiguous rows; on-chip layout is [channels->partitions, tokens->free],
    so conv taps are free-dim offsets.
  - HBM efficiency depends strongly on per-partition row length (2KB rows:
    ~250GB/s, 32KB rows: ~395GB/s with all 8 cores). So IO moves in
    [128, 8192] megatiles (32KB rows): one DMA in, one DMA out per half-group.
  - No single engine covers 4 fp32 taps under the HBM floor (~170us/core):
    fp32 PE matmul is 2-pass (~4cyc/col), fp32 DVE tensor-ops are 1x. Work is
    split by token range within each megatile:
      * 4 PE tiles (512 tok): 4 diagonal-matrix matmuls accumulate taps in
        PSUM; ScalarE silu(psum+bias) into the output megatile.
      * 3 DVE quads (2048 tok): ScalarE seeds tap0 (Copy with per-channel
        scale), VectorE chains 3 scalar_tensor_tensor fused MACs in-place,
        ScalarE silu in-place.
  - Sequence-boundary tokens (first 3 of each sequence, <= 27 rows) are
    recomputed exactly on the host; the conv-state cache update (1MB
    gather/scatter) is metadata-sized and also done on the host.
"""

import numpy as np

T = 16384
D = 4096
K = 4
NCORES = 8
DC = D // NCORES  # 512 channels per core
G = DC // 128     # 4 partition groups per core
HALO = K - 1      # 3

F_IO = 8192       # megatile tokens (32KB rows)
F_PE = 512        # PE tile (one fp32 PSUM bank)
F_DVE = 2048      # DVE quad
N_PE_HALF = 4     # PE tiles per megatile; rest is DVE quads
N_DVE_HALF = (F_IO - N_PE_HALF * F_PE) // F_DVE  # 3

_cached_nc = None


def _build_device_kernel():
    import concourse.bacc as bacc
    import concourse.mybir as mybir
    from concourse.masks import make_identity
    from concourse.tile import TileContext

    f32 = mybir.dt.float32
    mult = mybir.AluOpType.mult
    add = mybir.AluOpType.add
    silu_fn = mybir.ActivationFunctionType.Silu
    copy_fn = mybir.ActivationFunctionType.Copy

    nc = bacc.Bacc("TRN2", target_bir_lowering=False, debug=False,
                   num_devices=NCORES)

    xt = nc.dram_tensor("xt", [DC, T], f32, kind="ExternalInput")
    ws = nc.dram_tensor("ws", [128, G, K], f32, kind="ExternalInput")
    bs = nc.dram_tensor("bs", [128, G], f32, kind="ExternalInput")
    yt = nc.dram_tensor("yt", [DC, T], f32, kind="ExternalOutput")

    with TileContext(nc) as tc:
        with (
            tc.tile_pool(name="const", bufs=1) as cpool,
            tc.tile_pool(name="xb", bufs=2) as xpool,
            tc.tile_pool(name="yb", bufs=3) as ypool,
            tc.tile_pool(name="ps", bufs=8, space="PSUM") as ppool,
        ):
            ws_sb = cpool.tile([128, G, K], f32)
            nc.sync.dma_start(out=ws_sb[:], in_=ws[:])
            bs_sb = cpool.tile([128, G], f32)
            nc.sync.dma_start(out=bs_sb[:], in_=bs[:])
            ident = cpool.tile([128, 128], f32)
            make_identity(nc, ident[:])
            wd_sb = cpool.tile([128, G, K, 128], f32)
            for g in range(G):
                for j in range(K):
                    nc.vector.tensor_scalar_mul(
                        wd_sb[:, g, j, :], ident[:], ws_sb[:, g, j:j + 1])

            for g in range(G):
                rows = slice(g * 128, (g + 1) * 128)
                for t0 in range(0, T, F_IO):
                    xb = xpool.tile([128, F_IO + HALO], f32)
                    if t0 == 0:
                        nc.gpsimd.memset(xb[:, 0:HALO], 0.0)
                        nc.sync.dma_start(out=xb[:, HALO:], in_=xt[rows, 0:F_IO])
                    else:
                        nc.sync.dma_start(
                            out=xb[:], in_=xt[rows, t0 - HALO:t0 + F_IO])
                    yb = ypool.tile([128, F_IO], f32)

                    # --- DVE quads first (ACT seeds them early) ---
                    for q in range(N_DVE_HALF):
                        u = N_PE_HALF * F_PE + q * F_DVE
                        acc = yb[:, u:u + F_DVE]
                        nc.scalar.activation(
                            acc, xb[:, u:u + F_DVE], copy_fn,
                            bias=0.0, scale=ws_sb[:, g, 0:1])
                        for j in (1, 2, 3):
                            nc.vector.scalar_tensor_tensor(
                                out=acc, in0=xb[:, u + j:u + j + F_DVE],
                                scalar=ws_sb[:, g, j:j + 1], in1=acc,
                                op0=mult, op1=add)
                        nc.scalar.activation(
                            acc, acc, silu_fn,
                            bias=bs_sb[:, g:g + 1], scale=1.0)

                    # --- PE tiles (consecutive, so matmul passes pipeline) ---
                    for p in range(N_PE_HALF):
                        u = p * F_PE
                        ps = ppool.tile([128, F_PE], f32)
                        for j in range(K):
                            # tap j: out[c,t] += w[c,j] * x[c, t-(K-1-j)]
                            nc.tensor.matmul(
                                ps[:], wd_sb[:, g, j, :],
                                xb[:, u + j:u + j + F_PE],
                                start=(j == 0), stop=(j == K - 1),
                            )
                        nc.scalar.activation(
                            yb[:, u:u + F_PE], ps[:], silu_fn,
                            bias=bs_sb[:, g:g + 1], scale=1.0)

                    nc.sync.dma_start(out=yt[rows, t0:t0 + F_IO], in_=yb[:])

    nc.compile()
    return nc


def _get_nc():
    global _cached_nc
    if _cached_nc is None:
        _cached_nc = _build_device_kernel()
    return _cached_nc


def _silu(a):
    return a * (1.0 / (1.0 + np.exp(-a)))


def kernel(x, weight, bias, conv_state, seq_idx, conv_idx, state_ids,
           _run_opts=None):
    from concourse.bass_utils import run_bass_kernel_spmd

    x = np.asarray(x)
    weight = np.asarray(weight)
    bias = np.asarray(bias)
    conv_state = np.asarray(conv_state)
    seq_idx = np.asarray(seq_idx)
    conv_idx = np.asarray(conv_idx)
    state_ids = np.asarray(state_ids)

    x0 = x[0]                               # (T, D) f32
    w = weight[:, 0, :].astype(np.float32)  # (D, K)
    xT = np.ascontiguousarray(x0.T)         # (D, T)

    in_maps = []
    for c in range(NCORES):
        lo = c * DC
        w_core = w[lo:lo + DC]              # (DC, K)
        wsm = np.ascontiguousarray(
            w_core.reshape(G, 128, K).transpose(1, 0, 2))   # (128, G, K)
        bsm = np.ascontiguousarray(
            bias[lo:lo + DC].astype(np.float32).reshape(G, 128).T)  # (128, G)
        in_maps.append({
            "xt": np.ascontiguousarray(xT[lo:lo + DC]),
            "ws": wsm,
            "bs": bsm,
        })

    nc = _get_nc()
    run_opts = _run_opts or {}
    res = run_bass_kernel_spmd(nc, in_maps, core_ids=list(range(NCORES)),
                               **run_opts)

    outT = np.concatenate([r["yt"] for r in res.results], axis=0)  # (D, T)
    out = np.ascontiguousarray(outT.T)[None]                       # (1, T, D)

    # --- host fixup: first K-1 tokens of every sequence (exact recompute) ---
    starts = np.concatenate([[0], np.flatnonzero(np.diff(seq_idx) != 0) + 1])
    fix = (starts[:, None] + np.arange(HALO)[None]).ravel()
    fix = np.unique(fix[fix < T])
    if fix.size:
        acc = np.broadcast_to(bias.astype(np.float32), (fix.size, D)).copy()
        for j in range(K):
            s = K - 1 - j
            tm = fix - s
            tm_c = np.clip(tm, 0, T - 1)
            valid = (tm >= 0) & (seq_idx[tm_c] == seq_idx[fix])
            acc += np.where(valid[:, None], x0[tm_c], 0.0) * w[None, :, j]
        out[0, fix] = _silu(acc)

    # --- conv-state cache update (gather last-K rows, scatter into pool) ---
    new_conv_state = conv_state.copy()
    new_conv_state[state_ids] = np.transpose(x0[conv_idx], (0, 2, 1))

    if _run_opts is not None:
        return (out, new_conv_state), res
    return out, new_conv_state


# revision 14
# speedup vs baseline: 1.3193x; 1.0887x over previous
"""Causal depthwise conv1d (K=4) over packed ragged sequences + SiLU + conv-state
cache update, sharded channel-wise across 8 trn2 NeuronCores.

Strategy:
  - Channels (D=4096) sharded 512/core (tensor-parallel, per the module's tp
    logic). Host transposes x to channel-major (D, T) so each core DMAs
    contiguous rows; on-chip layout is [channels->partitions, tokens->free],
    so conv taps are free-dim offsets.
  - HBM efficiency depends strongly on per-partition row length (2KB rows:
    ~250GB/s, 32KB rows: ~395GB/s with all 8 cores). So IO moves in
    [128, 8192] megatiles (32KB rows): one DMA in, one DMA out per half-group.
  - No single engine covers 4 fp32 taps under the HBM floor (~170us/core):
    fp32 PE matmul is 2-pass (~4cyc/col), fp32 DVE tensor-ops are 1x. Work is
    split by token range within each megatile:
      * 4 PE tiles (512 tok): 4 diagonal-matrix matmuls accumulate taps in
        PSUM; ScalarE silu(psum+bias) into the output megatile.
      * 3 DVE quads (2048 tok): ScalarE seeds tap0 (Copy with per-channel
        scale), VectorE chains 3 scalar_tensor_tensor fused MACs in-place,
        ScalarE silu in-place.
  - Sequence-boundary tokens (first 3 of each sequence, <= 27 rows) are
    recomputed exactly on the host; the conv-state cache update (1MB
    gather/scatter) is metadata-sized and also done on the host.
"""

import numpy as np

T = 16384
D = 4096
K = 4
NCORES = 8
DC = D // NCORES  # 512 channels per core
G = DC // 128     # 4 partition groups per core
HALO = K - 1      # 3

F_IO = 4096       # megatile tokens (16KB rows)
F_PE = 512        # PE tile (one fp32 PSUM bank)
F_DVE = 2048      # DVE quad
# megatile types: 'A' = 4 PE tiles + 1 quad; 'C' = 2 quads
TILE_TYPES = ("A", "C", "A", "C")

_cached_nc = None


def _build_device_kernel():
    import concourse.bacc as bacc
    import concourse.mybir as mybir
    from concourse.masks import make_identity
    from concourse.tile import TileContext

    f32 = mybir.dt.float32
    mult = mybir.AluOpType.mult
    add = mybir.AluOpType.add
    silu_fn = mybir.ActivationFunctionType.Silu
    copy_fn = mybir.ActivationFunctionType.Copy

    nc = bacc.Bacc("TRN2", target_bir_lowering=False, debug=False,
                   num_devices=NCORES)

    xt = nc.dram_tensor("xt", [DC, T], f32, kind="ExternalInput")
    ws = nc.dram_tensor("ws", [128, G, K], f32, kind="ExternalInput")
    bs = nc.dram_tensor("bs", [128, G], f32, kind="ExternalInput")
    yt = nc.dram_tensor("yt", [DC, T], f32, kind="ExternalOutput")

    with TileContext(nc) as tc:
        with (
            tc.tile_pool(name="const", bufs=1) as cpool,
            tc.tile_pool(name="xb", bufs=4) as xpool,
            tc.tile_pool(name="yb", bufs=4) as ypool,
            tc.tile_pool(name="ps", bufs=8, space="PSUM") as ppool,
        ):
            ws_sb = cpool.tile([128, G, K], f32)
            nc.sync.dma_start(out=ws_sb[:], in_=ws[:])
            bs_sb = cpool.tile([128, G], f32)
            nc.sync.dma_start(out=bs_sb[:], in_=bs[:])
            ident = cpool.tile([128, 128], f32)
            make_identity(nc, ident[:])
            wd_sb = cpool.tile([128, G, K, 128], f32)
            for g in range(G):
                for j in range(K):
                    nc.vector.tensor_scalar_mul(
                        wd_sb[:, g, j, :], ident[:], ws_sb[:, g, j:j + 1])

            def emit_quad(g, xb, yb, u):
                acc = yb[:, u:u + F_DVE]
                nc.scalar.activation(
                    acc, xb[:, u:u + F_DVE], copy_fn,
                    bias=0.0, scale=ws_sb[:, g, 0:1])
                for j in (1, 2, 3):
                    nc.vector.scalar_tensor_tensor(
                        out=acc, in0=xb[:, u + j:u + j + F_DVE],
                        scalar=ws_sb[:, g, j:j + 1], in1=acc,
                        op0=mult, op1=add)
                nc.scalar.activation(
                    acc, acc, silu_fn, bias=bs_sb[:, g:g + 1], scale=1.0)

            def emit_pe(g, xb, yb, u):
                ps = ppool.tile([128, F_PE], f32)
                for j in range(K):
                    # tap j: out[c,t] += w[c,j] * x[c, t-(K-1-j)]
                    nc.tensor.matmul(
                        ps[:], wd_sb[:, g, j, :], xb[:, u + j:u + j + F_PE],
                        start=(j == 0), stop=(j == K - 1),
                    )
                nc.scalar.activation(
                    yb[:, u:u + F_PE], ps[:], silu_fn,
                    bias=bs_sb[:, g:g + 1], scale=1.0)

            for g in range(G):
                rows = slice(g * 128, (g + 1) * 128)
                for mt, typ in enumerate(TILE_TYPES):
                    t0 = mt * F_IO
                    xb = xpool.tile([128, F_IO + HALO], f32)
                    if t0 == 0:
                        nc.gpsimd.memset(xb[:, 0:HALO], 0.0)
                        nc.sync.dma_start(out=xb[:, HALO:], in_=xt[rows, 0:F_IO])
                    else:
                        nc.sync.dma_start(
                            out=xb[:], in_=xt[rows, t0 - HALO:t0 + F_IO])
                    yb = ypool.tile([128, F_IO], f32)

                    if typ == "A":
                        emit_quad(g, xb, yb, 4 * F_PE)
                        for p in range(4):
                            emit_pe(g, xb, yb, p * F_PE)
                    else:
                        emit_quad(g, xb, yb, 0)
                        emit_quad(g, xb, yb, F_DVE)

                    nc.sync.dma_start(out=yt[rows, t0:t0 + F_IO], in_=yb[:])

    nc.compile()
    return nc


def _get_nc():
    global _cached_nc
    if _cached_nc is None:
        _cached_nc = _build_device_kernel()
    return _cached_nc


def _silu(a):
    return a * (1.0 / (1.0 + np.exp(-a)))


def kernel(x, weight, bias, conv_state, seq_idx, conv_idx, state_ids,
           _run_opts=None):
    from concourse.bass_utils import run_bass_kernel_spmd

    x = np.asarray(x)
    weight = np.asarray(weight)
    bias = np.asarray(bias)
    conv_state = np.asarray(conv_state)
    seq_idx = np.asarray(seq_idx)
    conv_idx = np.asarray(conv_idx)
    state_ids = np.asarray(state_ids)

    x0 = x[0]                               # (T, D) f32
    w = weight[:, 0, :].astype(np.float32)  # (D, K)
    xT = np.ascontiguousarray(x0.T)         # (D, T)

    in_maps = []
    for c in range(NCORES):
        lo = c * DC
        w_core = w[lo:lo + DC]              # (DC, K)
        wsm = np.ascontiguousarray(
            w_core.reshape(G, 128, K).transpose(1, 0, 2))   # (128, G, K)
        bsm = np.ascontiguousarray(
            bias[lo:lo + DC].astype(np.float32).reshape(G, 128).T)  # (128, G)
        in_maps.append({
            "xt": np.ascontiguousarray(xT[lo:lo + DC]),
            "ws": wsm,
            "bs": bsm,
        })

    nc = _get_nc()
    run_opts = _run_opts or {}
    res = run_bass_kernel_spmd(nc, in_maps, core_ids=list(range(NCORES)),
                               **run_opts)

    outT = np.concatenate([r["yt"] for r in res.results], axis=0)  # (D, T)
    out = np.ascontiguousarray(outT.T)[None]                       # (1, T, D)

    # --- host fixup: first K-1 tokens of every sequence (exact recompute) ---
    starts = np.concatenate([[0], np.flatnonzero(np.diff(seq_idx) != 0) + 1])
    fix = (starts[:, None] + np.arange(HALO)[None]).ravel()
    fix = np.unique(fix[fix < T])
    if fix.size:
        acc = np.broadcast_to(bias.astype(np.float32), (fix.size, D)).copy()
        for j in range(K):
            s = K - 1 - j
            tm = fix - s
            tm_c = np.clip(tm, 0, T - 1)
            valid = (tm >= 0) & (seq_idx[tm_c] == seq_idx[fix])
            acc += np.where(valid[:, None], x0[tm_c], 0.0) * w[None, :, j]
        out[0, fix] = _silu(acc)

    # --- conv-state cache update (gather last-K rows, scatter into pool) ---
    new_conv_state = conv_state.copy()
    new_conv_state[state_ids] = np.transpose(x0[conv_idx], (0, 2, 1))

    if _run_opts is not None:
        return (out, new_conv_state), res
    return out, new_conv_state
